# revision 19
# baseline (speedup 1.0000x reference)
"""BatchHardTripletLoss on 8 Trainium2 NeuronCores — flipped + norm-dealt.

Layout: rows label-sorted on host; each core streams its 1024 anchors
(free dim) against all B=8192 embeddings as 64 column chunks of 128
(partition dim), processed as 32 psum pairs [128, 2048].

  - Own pairs (K/2 ~ 5): chunks holding the core's own labels (plus
    fillers). Raw -2x.x tiles are ACT-copied to f16 and shipped to the
    HOST over the idle DMA path; the host adds exact sq_j, masks
    same-label/self pairs, and computes hardest-pos plus the own-side
    hardest-neg. No penalty matmuls, no own-side DVE work.
  - Dealt pairs: remaining columns norm-sorted and dealt so sq_j is
    near-constant per partition. Three consumer paths, balanced across
    engines: (A) ACT Copy + one wide f16 chain min on DVE; (B) strided
    DVE pair-min reduce straight from psum into a side chain; (C) ACT
    Copy + ship to host (exact sq there). The per-partition bias s_hat
    applies once via a fused STT; the last pair takes path B after
    everything else is pre-merged, keeping the critical tail short.
  - Partition-direction hn min via PE transposes + DVE free-dim reduces;
    sqrt/relu/mean and the final combine on host.
"""

import sys

import numpy as np

if "/opt/trn_rl_repo" not in sys.path:
    sys.path.insert(0, "/opt/trn_rl_repo")

from concourse import bacc, bass, mybir, tile
from concourse.bass_utils import run_bass_kernel_spmd

B = 8192
D = 128
C = 128
N_CORES = 8
R = B // N_CORES          # anchors per core
NCH = B // 128            # column chunks (64)
RT = R // 128             # anchor blocks for the tail transposes (8)
PEN = 1024.0

F16 = mybir.dt.float16
F32 = mybir.dt.float32
ALU = mybir.AluOpType
ACTF = mybir.ActivationFunctionType

_NC_CACHE = {}


def _paths(npair):
    bset = {4, 8, 13, 17, 22, npair - 1}
    cset = {2, 6, 10, 15, 19, 24} - bset
    return bset, cset


def _build_nc(kown):
    npair = (NCH - kown) // 2
    nown = kown // 2
    bset, cset = _paths(npair)
    clist = sorted(cset)
    nc = bacc.Bacc(None, target_bir_lowering=False)

    xt_d = nc.declare_dram_parameter("xt", [128, B], F16, isOutput=False)
    xls_d = nc.declare_dram_parameter("xls", [128, R], F16, isOutput=False)
    shat_d = nc.declare_dram_parameter("shat", [128, 1], F32, isOutput=False)
    idn_d = nc.declare_dram_parameter("idn", [128, 128], F16, isOutput=False)
    hn2_d = nc.declare_dram_parameter("hn2", [128, RT], F32, isOutput=True)
    hop_d = nc.declare_dram_parameter("hop", [128, nown * 2048], F16, isOutput=True)
    hsp_d = nc.declare_dram_parameter("hsp", [128, len(cset) * 2048], F16,
                                      isOutput=True)

    with tile.TileContext(nc) as tc:
        with tc.tile_pool(name="const", bufs=1) as cp:
            XTS = [cp.tile([128, 1024], F16, name=f"xts{s}") for s in range(8)]
            XLS = cp.tile([128, R], F16)
            SHAT = cp.tile([128, 1], F32)
            IDN = cp.tile([128, 128], F16)
            ACC2 = [cp.tile([128, 2048], F16, name=f"acc{k}") for k in range(2)]
            WACC = [cp.tile([128, R], F16, name=f"wacc{k}") for k in range(2)]
            PRE = cp.tile([128, R], F16)
            ACCF = cp.tile([128, R], F16)
            DUM = cp.tile([128, R], F16)
            HN2 = cp.tile([128, RT], F32)

            # first dealt pair needs only XLS halves + one slice of XT;
            # split those DMAs so compute starts as early as possible
            nc.scalar.dma_start(XLS[:, 0:512], xls_d[:, 0:512])
            nc.scalar.dma_start(XLS[:, 512:1024], xls_d[:, 512:1024])
            nc.scalar.dma_start(SHAT[:], shat_d[:])
            nc.sync.dma_start(XTS[1][:, 256:512], xt_d[:, 1280:1536])
            nc.sync.dma_start(XTS[1][:, 0:256], xt_d[:, 1024:1280])
            nc.sync.dma_start(XTS[1][:, 512:1024], xt_d[:, 1536:2048])
            nc.sync.dma_start(XTS[0][:], xt_d[:, 0:1024])
            nc.sync.dma_start(XTS[2][:], xt_d[:, 2048:3072])
            for s in range(3, 8):
                nc.sync.dma_start(XTS[s][:], xt_d[:, s * 1024 : (s + 1) * 1024])
            nc.sync.dma_start(IDN[:], idn_d[:])
            nc.vector.memset(ACC2[0][:], 60000.0)
            nc.vector.memset(DUM[:], 60000.0)

            def chunk_lhs(ch):
                return XTS[ch // 8][:, (ch % 8) * 128 : (ch % 8) * 128 + 128]

            sched = [("d", 0), ("d", 1)]
            sched += [("o", u) for u in range(nown)]
            sched += [("d", t) for t in range(2, npair - 1)]

            ia = 0   # ACC2 ping-pong
            iw = -1  # WACC state (-1: empty)
            nship = 0

            with (
                tc.tile_pool(name="dpsum", bufs=2, space=bass.MemorySpace.PSUM) as pd,
                tc.tile_pool(name="opool", bufs=4) as op,
                tc.tile_pool(name="gpool", bufs=4) as gp,
                tc.tile_pool(name="tpool", bufs=2) as tp,
            ):
                def emit_mms(ps, kind, t):
                    for j in range(2):
                        lhs = chunk_lhs((2 * t + j) if kind == "o"
                                        else kown + 2 * t + j)
                        for h in range(2):
                            sl = slice(j * 1024 + h * 512, j * 1024 + (h + 1) * 512)
                            nc.tensor.matmul(ps[:, sl], lhs,
                                             XLS[:, h * 512 : (h + 1) * 512],
                                             start=True, stop=True)

                for kind, t in sched:
                    ps = pd.tile([128, 2048], F32, tag="ps")
                    emit_mms(ps, kind, t)
                    if kind == "o":
                        T2 = op.tile([128, 2048], F16, tag="t2")
                        nc.scalar.activation(T2[:], ps[:], ACTF.Copy)
                        nc.sync.dma_start(
                            hop_d[:, t * 2048 : (t + 1) * 2048], T2[:])
                    elif t in bset:
                        # strided pair-min straight from psum into a side chain
                        if iw < 0:
                            nc.vector.tensor_reduce(
                                WACC[0][:], ps[:].rearrange("p (c i) -> p i c", c=2),
                                axis=mybir.AxisListType.X, op=ALU.min)
                            iw = 0
                        else:
                            W = tp.tile([128, R], F16, tag="w")
                            nc.vector.tensor_reduce(
                                W[:], ps[:].rearrange("p (c i) -> p i c", c=2),
                                axis=mybir.AxisListType.X, op=ALU.min)
                            nc.vector.tensor_tensor(
                                WACC[1 - iw][:], WACC[iw][:], W[:], op=ALU.min)
                            iw = 1 - iw
                    elif t in cset:
                        # ship raw f16 pair to the host (exact sq there)
                        G = gp.tile([128, 2048], F16, tag="g")
                        nc.scalar.activation(G[:], ps[:], ACTF.Copy)
                        nc.sync.dma_start(
                            hsp_d[:, nship * 2048 : (nship + 1) * 2048], G[:])
                        nship += 1
                    else:
                        # wide chain: one [2048] min per pair
                        G = gp.tile([128, 2048], F16, tag="g")
                        nc.scalar.activation(G[:], ps[:], ACTF.Copy)
                        nc.vector.tensor_tensor(
                            ACC2[1 - ia][:], ACC2[ia][:], G[:], op=ALU.min)
                        ia = 1 - ia

                # pre-merge main chain + side chain before the last pair
                MF = tp.tile([128, R], F16, tag="mf")
                nc.vector.tensor_tensor(MF[:], ACC2[ia][:, 0:1024],
                                        ACC2[ia][:, 1024:2048], op=ALU.min)
                nc.vector.tensor_tensor(PRE[:], MF[:], WACC[iw][:], op=ALU.min)
                # last dealt pair: strided reduce, single merge, biased STT
                ps = pd.tile([128, 2048], F32, tag="ps")
                emit_mms(ps, "d", npair - 1)
                WL = tp.tile([128, R], F16, tag="wl")
                nc.vector.tensor_reduce(
                    WL[:], ps[:].rearrange("p (c i) -> p i c", c=2),
                    axis=mybir.AxisListType.X, op=ALU.min)
                FD = tp.tile([128, R], F16, tag="fd")
                nc.vector.tensor_tensor(FD[:], PRE[:], WL[:], op=ALU.min)
                nc.vector.scalar_tensor_tensor(
                    ACCF[:], FD[:], SHAT[:, 0:1], DUM[:],
                    op0=ALU.add, op1=ALU.min,
                )

            with tc.tile_pool(name="fpsum", bufs=4, space=bass.MemorySpace.PSUM) as pf:
                for t in range(RT):
                    pn = pf.tile([128, 128], F16, tag="pn")
                    nc.tensor.transpose(pn[:], ACCF[:, t * 128 : (t + 1) * 128], IDN[:])
                    nc.vector.tensor_reduce(HN2[:, t : t + 1], pn[:],
                                            axis=mybir.AxisListType.X, op=ALU.min)

            nc.sync.dma_start(hn2_d[:], HN2[:])

    nc.compile()
    return nc


def _get_nc(kown):
    if kown not in _NC_CACHE:
        _NC_CACHE[kown] = _build_nc(kown)
    return _NC_CACHE[kown]


def _prep_in_maps(embeddings, labels):
    x = np.asarray(embeddings, dtype=np.float32)
    lab = np.asarray(labels).astype(np.int64)
    order = np.argsort(lab, kind="stable")
    lab_s = lab[order]
    xs = x[order]
    xt = np.ascontiguousarray(xs.T).astype(np.float16)   # [128, B]
    sq = (xs.astype(np.float64) ** 2).sum(1).astype(np.float32)
    idn = np.eye(128, dtype=np.float16)
    own_sets = []
    K = 0
    for m in range(N_CORES):
        mylab = lab_s[m * R : (m + 1) * R]
        own_idx = np.flatnonzero((lab_s >= mylab.min()) & (lab_s <= mylab.max()))
        own_sets.append(own_idx)
        K = max(K, -(-len(own_idx) // 128))
    K += K % 2
    in_maps = []
    extras = []
    for m in range(N_CORES):
        own_idx = own_sets[m]
        mask = np.zeros(B, bool)
        mask[own_idx] = True
        non_own = np.flatnonzero(~mask)
        n_fill = K * 128 - len(own_idx)
        fill, dealt = non_own[:n_fill], non_own[n_fill:]
        own_cols = np.concatenate([own_idx, fill])
        dsort = dealt[np.argsort(sq[dealt], kind="stable")]
        deal_mat = dsort.reshape(128, NCH - K)           # [partition, chunk]
        cols = np.concatenate([own_cols, deal_mat.T.reshape(-1)])
        in_maps.append({
            "xt": np.ascontiguousarray(xt[:, cols]),
            "xls": np.ascontiguousarray(
                (-2.0 * xs[m * R : (m + 1) * R].T)).astype(np.float16),
            "shat": sq[deal_mat].mean(1, dtype=np.float64).astype(np.float32).reshape(128, 1),
            "idn": idn,
        })
        extras.append((own_cols, deal_mat))
    return in_maps, lab, order, lab_s, sq, K, extras


def run_cores(embeddings, labels, trace=False, **kw):
    in_maps, lab, order, lab_s, sq, K, extras = _prep_in_maps(embeddings, labels)
    nc = _get_nc(K)
    npair = (NCH - K) // 2
    bset, cset = _paths(npair)
    clist = sorted(cset)
    res = run_bass_kernel_spmd(nc, in_maps, list(range(N_CORES)), trace=trace, **kw)
    hn2_parts, hp2_parts = [], []
    for m, r in enumerate(res.results):
        own_cols, deal_mat = extras[m]
        hn2 = np.asarray(r["hn2"], np.float32).T.reshape(R)
        # own tiles: exact host min/max with masking
        V = np.asarray(r["hop"], np.float32).reshape(128, K // 2, 2, R)
        V = V.transpose(1, 2, 0, 3).reshape(K * 128, R)
        D2 = V + sq[own_cols][:, None]
        mylab = lab_s[m * R : (m + 1) * R]
        same = lab_s[own_cols][:, None] == mylab[None, :]
        selfm = own_cols[:, None] == (m * R + np.arange(R))[None, :]
        hn2 = np.minimum(hn2, np.where(~same, D2, np.inf).min(0))
        hp2 = np.where(same & ~selfm, D2, -np.inf).max(0)
        # shipped dealt pairs: exact sq on host
        S = np.asarray(r["hsp"], np.float32).reshape(128, len(clist), 2, R)
        ship_cols = np.stack(
            [deal_mat[:, [2 * t, 2 * t + 1]] for t in clist])  # [nc, 128, 2]
        Ssq = sq[ship_cols].transpose(1, 0, 2)                 # [128, nc, 2]
        hn_ship = (S + Ssq[:, :, :, None]).min(axis=(0, 1, 2))
        hn2 = np.minimum(hn2, hn_ship)
        hn2_parts.append(hn2)
        hp2_parts.append(hp2)
    hn2 = np.concatenate(hn2_parts)
    hp2 = np.concatenate(hp2_parts)
    hn = np.sqrt(np.maximum(hn2 + sq, 0.0))
    hp = np.sqrt(np.maximum(np.where(np.isfinite(hp2), hp2, -sq) + sq, 0.0))
    pr_sorted = np.maximum(hp - hn + 1.0, 0.0)
    pr = np.empty(B, np.float32)
    pr[order] = pr_sorted
    counts = np.bincount(lab, minlength=C)
    valid = (counts[lab] >= 2) & (counts[lab] <= B - 1)
    nv = int(valid.sum())
    loss = float((pr * valid).sum() / nv) if nv > 0 else 0.0
    return np.float32(loss), res


def kernel(embeddings, labels):
    loss, _ = run_cores(embeddings, labels, trace=False)
    return loss


# revision 20
# speedup vs baseline: 1.0565x; 1.0565x over previous
"""BatchHardTripletLoss on 8 Trainium2 NeuronCores — flipped + norm-dealt.

Layout: rows label-sorted on host; each core streams its 1024 anchors
(free dim) against all B=8192 embeddings as 64 column chunks of 128
(partition dim), processed as 32 psum pairs [128, 2048].

  - Own pairs (K/2 ~ 5): chunks holding the core's own labels (plus
    fillers), spaced through the schedule. Raw -2x.x tiles are ACT-copied
    to f16 and shipped to the HOST over the idle DMA path; the host adds
    exact sq_j, masks same-label/self pairs, and computes hardest-pos
    plus the own-side hardest-neg. No penalty matmuls, no own DVE work.
  - Dealt pairs: remaining columns norm-sorted and dealt so sq_j is
    near-constant per partition. Two paths balanced across engines:
    (A) ACT Copy + one wide [2048] f16 chain min on DVE; (B) one direct
    chain min against the f32 psum pair on DVE (no ACT). The
    per-partition bias s_hat applies once via a fused STT; the last pair
    is path B after a pre-fold so the critical tail stays short.
  - Partition-direction hn min via PE transposes + DVE free-dim reduces;
    sqrt/relu/mean and the final combine on host.
"""

import sys

import numpy as np

if "/opt/trn_rl_repo" not in sys.path:
    sys.path.insert(0, "/opt/trn_rl_repo")

from concourse import bacc, bass, mybir, tile
from concourse.bass_utils import run_bass_kernel_spmd

B = 8192
D = 128
C = 128
N_CORES = 8
R = B // N_CORES          # anchors per core
NCH = B // 128            # column chunks (64)
RT = R // 128             # anchor blocks for the tail transposes (8)
PEN = 1024.0

F16 = mybir.dt.float16
F32 = mybir.dt.float32
ALU = mybir.AluOpType
ACTF = mybir.ActivationFunctionType

_NC_CACHE = {}


def _build_nc(kown):
    npair = (NCH - kown) // 2
    nown = kown // 2
    bset = {3, 7, 11, 15, 19, 23, npair - 1}
    nc = bacc.Bacc(None, target_bir_lowering=False)

    xt_d = nc.declare_dram_parameter("xt", [128, B], F16, isOutput=False)
    xls_d = nc.declare_dram_parameter("xls", [128, R], F16, isOutput=False)
    shat_d = nc.declare_dram_parameter("shat", [128, 1], F32, isOutput=False)
    idn_d = nc.declare_dram_parameter("idn", [128, 128], F16, isOutput=False)
    hn2_d = nc.declare_dram_parameter("hn2", [128, RT], F32, isOutput=True)
    hop_d = nc.declare_dram_parameter("hop", [128, nown * 2048], F16, isOutput=True)

    with tile.TileContext(nc) as tc:
        with tc.tile_pool(name="const", bufs=1) as cp:
            XTS = [cp.tile([128, 1024], F16, name=f"xts{s}") for s in range(8)]
            XLS = cp.tile([128, R], F16)
            SHAT = cp.tile([128, 1], F32)
            IDN = cp.tile([128, 128], F16)
            ACC2 = [cp.tile([128, 2048], F16, name=f"acc{k}") for k in range(2)]
            PRE = cp.tile([128, R], F16)
            ACCF = cp.tile([128, R], F16)
            DUM = cp.tile([128, R], F16)
            HN2 = cp.tile([128, RT], F32)

            # first dealt pair needs only XLS + one quarter-slice of XT
            nc.scalar.dma_start(XLS[:, 0:512], xls_d[:, 0:512])
            nc.scalar.dma_start(XLS[:, 512:1024], xls_d[:, 512:1024])
            nc.scalar.dma_start(SHAT[:], shat_d[:])
            nc.sync.dma_start(XTS[1][:, 256:512], xt_d[:, 1280:1536])
            nc.sync.dma_start(XTS[1][:, 0:256], xt_d[:, 1024:1280])
            nc.sync.dma_start(XTS[1][:, 512:1024], xt_d[:, 1536:2048])
            nc.sync.dma_start(XTS[0][:], xt_d[:, 0:1024])
            nc.sync.dma_start(XTS[2][:], xt_d[:, 2048:3072])
            for s in range(3, 8):
                nc.sync.dma_start(XTS[s][:], xt_d[:, s * 1024 : (s + 1) * 1024])
            nc.sync.dma_start(IDN[:], idn_d[:])
            nc.vector.memset(ACC2[0][:], 60000.0)
            nc.vector.memset(DUM[:], 60000.0)

            def chunk_lhs(ch):
                return XTS[ch // 8][:, (ch % 8) * 128 : (ch % 8) * 128 + 128]

            # spread the own pairs so their host-ship DMAs don't cluster
            sched = []
            dq = [("d", t) for t in range(npair - 1)]
            oq = [("o", u) for u in range(nown)]
            di = 0
            for i in range(len(dq) + len(oq)):
                if oq and di >= 2 and (di - 2) % 5 == 0 and i > 0 and sched[-1][0] == "d":
                    sched.append(oq.pop(0))
                elif dq:
                    sched.append(dq.pop(0))
                    di += 1
                else:
                    sched.append(oq.pop(0))

            ia = 0   # ACC2 ping-pong

            with (
                tc.tile_pool(name="dpsum", bufs=2, space=bass.MemorySpace.PSUM) as pd,
                tc.tile_pool(name="opool", bufs=3) as op,
                tc.tile_pool(name="gpool", bufs=4) as gp,
                tc.tile_pool(name="tpool", bufs=2) as tp,
            ):
                def emit_mms(ps, kind, t):
                    for j in range(2):
                        lhs = chunk_lhs((2 * t + j) if kind == "o"
                                        else kown + 2 * t + j)
                        for h in range(2):
                            sl = slice(j * 1024 + h * 512, j * 1024 + (h + 1) * 512)
                            nc.tensor.matmul(ps[:, sl], lhs,
                                             XLS[:, h * 512 : (h + 1) * 512],
                                             start=True, stop=True)

                for kind, t in sched:
                    ps = pd.tile([128, 2048], F32, tag="ps")
                    emit_mms(ps, kind, t)
                    if kind == "o":
                        T2 = op.tile([128, 2048], F16, tag="t2")
                        nc.scalar.activation(T2[:], ps[:], ACTF.Copy)
                        nc.sync.dma_start(
                            hop_d[:, t * 2048 : (t + 1) * 2048], T2[:])
                    elif t in bset:
                        # direct wide chain min against the f32 psum pair
                        nc.vector.tensor_tensor(
                            ACC2[1 - ia][:], ACC2[ia][:], ps[:], op=ALU.min)
                        ia = 1 - ia
                    else:
                        G = gp.tile([128, 2048], F16, tag="g")
                        nc.scalar.activation(G[:], ps[:], ACTF.Copy)
                        nc.vector.tensor_tensor(
                            ACC2[1 - ia][:], ACC2[ia][:], G[:], op=ALU.min)
                        ia = 1 - ia

                # pre-fold the wide chain before the last pair
                nc.vector.tensor_tensor(PRE[:], ACC2[ia][:, 0:1024],
                                        ACC2[ia][:, 1024:2048], op=ALU.min)
                ps = pd.tile([128, 2048], F32, tag="ps")
                emit_mms(ps, "d", npair - 1)
                X1 = tp.tile([128, R], F16, tag="x1")
                nc.vector.tensor_tensor(X1[:], PRE[:], ps[:, 0:1024], op=ALU.min)
                X2 = tp.tile([128, R], F16, tag="x2")
                nc.vector.tensor_tensor(X2[:], X1[:], ps[:, 1024:2048], op=ALU.min)
                nc.vector.scalar_tensor_tensor(
                    ACCF[:], X2[:], SHAT[:, 0:1], DUM[:],
                    op0=ALU.add, op1=ALU.min,
                )

            with tc.tile_pool(name="fpsum", bufs=4, space=bass.MemorySpace.PSUM) as pf:
                for t in range(RT):
                    pn = pf.tile([128, 128], F16, tag="pn")
                    nc.tensor.transpose(pn[:], ACCF[:, t * 128 : (t + 1) * 128], IDN[:])
                    nc.vector.tensor_reduce(HN2[:, t : t + 1], pn[:],
                                            axis=mybir.AxisListType.X, op=ALU.min)

            nc.sync.dma_start(hn2_d[:], HN2[:])

    nc.compile()
    return nc


def _get_nc(kown):
    if kown not in _NC_CACHE:
        _NC_CACHE[kown] = _build_nc(kown)
    return _NC_CACHE[kown]


def _prep_in_maps(embeddings, labels):
    x = np.asarray(embeddings, dtype=np.float32)
    lab = np.asarray(labels).astype(np.int64)
    order = np.argsort(lab, kind="stable")
    lab_s = lab[order]
    xs = x[order]
    xt = np.ascontiguousarray(xs.T).astype(np.float16)   # [128, B]
    sq = (xs.astype(np.float64) ** 2).sum(1).astype(np.float32)
    idn = np.eye(128, dtype=np.float16)
    own_sets = []
    K = 0
    for m in range(N_CORES):
        mylab = lab_s[m * R : (m + 1) * R]
        own_idx = np.flatnonzero((lab_s >= mylab.min()) & (lab_s <= mylab.max()))
        own_sets.append(own_idx)
        K = max(K, -(-len(own_idx) // 128))
    K += K % 2
    in_maps = []
    extras = []
    for m in range(N_CORES):
        own_idx = own_sets[m]
        mask = np.zeros(B, bool)
        mask[own_idx] = True
        non_own = np.flatnonzero(~mask)
        n_fill = K * 128 - len(own_idx)
        fill, dealt = non_own[:n_fill], non_own[n_fill:]
        own_cols = np.concatenate([own_idx, fill])
        dsort = dealt[np.argsort(sq[dealt], kind="stable")]
        deal_mat = dsort.reshape(128, NCH - K)           # [partition, chunk]
        cols = np.concatenate([own_cols, deal_mat.T.reshape(-1)])
        in_maps.append({
            "xt": np.ascontiguousarray(xt[:, cols]),
            "xls": np.ascontiguousarray(
                (-2.0 * xs[m * R : (m + 1) * R].T)).astype(np.float16),
            "shat": sq[deal_mat].mean(1, dtype=np.float64).astype(np.float32).reshape(128, 1),
            "idn": idn,
        })
        extras.append(own_cols)
    return in_maps, lab, order, lab_s, sq, K, extras


def run_cores(embeddings, labels, trace=False, **kw):
    in_maps, lab, order, lab_s, sq, K, extras = _prep_in_maps(embeddings, labels)
    nc = _get_nc(K)
    res = run_bass_kernel_spmd(nc, in_maps, list(range(N_CORES)), trace=trace, **kw)
    hn2_parts, hp2_parts = [], []
    for m, r in enumerate(res.results):
        own_cols = extras[m]
        hn2 = np.asarray(r["hn2"], np.float32).T.reshape(R)
        V = np.asarray(r["hop"], np.float32).reshape(128, K // 2, 2, R)
        V = V.transpose(1, 2, 0, 3).reshape(K * 128, R)
        D2 = V + sq[own_cols][:, None]
        mylab = lab_s[m * R : (m + 1) * R]
        same = lab_s[own_cols][:, None] == mylab[None, :]
        selfm = own_cols[:, None] == (m * R + np.arange(R))[None, :]
        hn2 = np.minimum(hn2, np.where(~same, D2, np.inf).min(0))
        hp2 = np.where(same & ~selfm, D2, -np.inf).max(0)
        hn2_parts.append(hn2)
        hp2_parts.append(hp2)
    hn2 = np.concatenate(hn2_parts)
    hp2 = np.concatenate(hp2_parts)
    hn = np.sqrt(np.maximum(hn2 + sq, 0.0))
    hp = np.sqrt(np.maximum(np.where(np.isfinite(hp2), hp2, -sq) + sq, 0.0))
    pr_sorted = np.maximum(hp - hn + 1.0, 0.0)
    pr = np.empty(B, np.float32)
    pr[order] = pr_sorted
    counts = np.bincount(lab, minlength=C)
    valid = (counts[lab] >= 2) & (counts[lab] <= B - 1)
    nv = int(valid.sum())
    loss = float((pr * valid).sum() / nv) if nv > 0 else 0.0
    return np.float32(loss), res


def kernel(embeddings, labels):
    loss, _ = run_cores(embeddings, labels, trace=False)
    return loss


# revision 24
# speedup vs baseline: 1.1795x; 1.1165x over previous
"""BatchHardTripletLoss on 8 Trainium2 NeuronCores — flipped + norm-dealt.

Layout: rows label-sorted on host; each core streams its 1024 anchors
(free dim) against all B=8192 embeddings as 64 column chunks of 128
(partition dim), processed as 32 psum pairs [128, 2048].

  - Own pairs (K/2 ~ 5): chunks holding the core's own labels (plus
    fillers), spaced through the schedule. Raw -2x.x tiles are ACT-copied
    to f16 and shipped to the HOST over the idle DMA path; the host adds
    exact sq_j, masks same-label/self pairs, and computes hardest-pos
    plus the own-side hardest-neg. No penalty matmuls, no own DVE work.
  - Dealt pairs: remaining columns norm-sorted and dealt so sq_j is
    near-constant per partition. Two paths balanced across engines:
    (A) ACT Copy + one wide [2048] f16 chain min on DVE; (B) one direct
    chain min against the f32 psum pair on DVE (no ACT). The
    per-partition bias s_hat applies once via a fused STT; the last pair
    is path B after a pre-fold so the critical tail stays short.
  - Partition-direction hn min via PE transposes + DVE free-dim reduces;
    sqrt/relu/mean and the final combine on host.
"""

import sys

import numpy as np

if "/opt/trn_rl_repo" not in sys.path:
    sys.path.insert(0, "/opt/trn_rl_repo")

from concourse import bacc, bass, mybir, tile
from concourse.bass_utils import run_bass_kernel_spmd

B = 8192
D = 128
C = 128
N_CORES = 8
R = B // N_CORES          # anchors per core
NCH = B // 128            # column chunks (64)
RT = R // 128             # anchor blocks for the tail transposes (8)
PEN = 1024.0

F16 = mybir.dt.float16
F32 = mybir.dt.float32
ALU = mybir.AluOpType
ACTF = mybir.ActivationFunctionType

_NC_CACHE = {}


def _build_nc(kown):
    npair = (NCH - kown) // 2
    nown = kown // 2
    bset = {3, 7, 11, 15, 19, 23, npair - 1}
    nc = bacc.Bacc(None, target_bir_lowering=False)

    xt_d = nc.declare_dram_parameter("xt", [128, B], F16, isOutput=False)
    xls_d = nc.declare_dram_parameter("xls", [128, R], F16, isOutput=False)
    shat_d = nc.declare_dram_parameter("shat", [128, 1], F32, isOutput=False)
    idn_d = nc.declare_dram_parameter("idn", [128, 128], F16, isOutput=False)
    hn2_d = nc.declare_dram_parameter("hn2", [128, RT], F32, isOutput=True)
    hop_d = nc.declare_dram_parameter("hop", [128, nown * 2048], F16, isOutput=True)

    with tile.TileContext(nc) as tc:
        with tc.tile_pool(name="const", bufs=1) as cp:
            XTS = [cp.tile([128, 1024], F16, name=f"xts{s}") for s in range(8)]
            XLS = cp.tile([128, R], F16)
            SHAT = cp.tile([128, 1], F32)
            IDN = cp.tile([128, 128], F16)
            ACCF = cp.tile([128, R], F16)
            DUM = cp.tile([128, R], F16)
            HN2 = cp.tile([128, RT], F32)

            # first dealt pair needs only XLS + one quarter-slice of XT
            nc.scalar.dma_start(XLS[:, 0:512], xls_d[:, 0:512])
            nc.scalar.dma_start(XLS[:, 512:1024], xls_d[:, 512:1024])
            nc.scalar.dma_start(SHAT[:], shat_d[:])
            nc.sync.dma_start(XTS[1][:, 256:512], xt_d[:, 1280:1536])
            nc.sync.dma_start(XTS[1][:, 0:256], xt_d[:, 1024:1280])
            nc.sync.dma_start(XTS[1][:, 512:1024], xt_d[:, 1536:2048])
            nc.sync.dma_start(XTS[0][:], xt_d[:, 0:1024])
            nc.sync.dma_start(XTS[2][:], xt_d[:, 2048:3072])
            for s in range(3, 8):
                nc.sync.dma_start(XTS[s][:], xt_d[:, s * 1024 : (s + 1) * 1024])
            nc.sync.dma_start(IDN[:], idn_d[:])
            nc.vector.memset(DUM[:], 60000.0)

            def chunk_lhs(ch):
                return XTS[ch // 8][:, (ch % 8) * 128 : (ch % 8) * 128 + 128]

            # spread the own pairs so their host-ship DMAs don't cluster
            sched = []
            dq = [("d", t) for t in range(npair - 1)]
            oq = [("o", u) for u in range(nown)]
            di = 0
            for i in range(len(dq) + len(oq)):
                if oq and di >= 2 and (di - 2) % 5 == 0 and i > 0 and sched[-1][0] == "d":
                    sched.append(oq.pop(0))
                elif dq:
                    sched.append(dq.pop(0))
                    di += 1
                else:
                    sched.append(oq.pop(0))

            levels = {}

            def tree_push(level, write_fn, tp):
                buf = levels.get(level)
                if buf is None:
                    nb = tp.tile([128, 2048], F16, tag=f"tr{level}")
                    write_fn(nb[:, 0:1024])
                    levels[level] = nb
                else:
                    write_fn(buf[:, 1024:2048])
                    levels[level] = None
                    tree_push(level + 1,
                              lambda dst, b=buf: nc.vector.tensor_tensor(
                                  dst, b[:, 0:1024], b[:, 1024:2048], op=ALU.min),
                              tp)

            with (
                tc.tile_pool(name="dpsum", bufs=2, space=bass.MemorySpace.PSUM) as pd,
                tc.tile_pool(name="opool", bufs=3) as op,
                tc.tile_pool(name="gpool", bufs=4) as gp,
                tc.tile_pool(name="tpool", bufs=2) as tp,
            ):
                def emit_mms(ps, kind, t):
                    for j in range(2):
                        lhs = chunk_lhs((2 * t + j) if kind == "o"
                                        else kown + 2 * t + j)
                        for h in range(2):
                            sl = slice(j * 1024 + h * 512, j * 1024 + (h + 1) * 512)
                            nc.tensor.matmul(ps[:, sl], lhs,
                                             XLS[:, h * 512 : (h + 1) * 512],
                                             start=True, stop=True)

                for kind, t in sched:
                    ps = pd.tile([128, 2048], F32, tag="ps")
                    emit_mms(ps, kind, t)
                    if kind == "o":
                        T2 = op.tile([128, 2048], F16, tag="t2")
                        nc.scalar.activation(T2[:], ps[:], ACTF.Copy)
                        nc.sync.dma_start(
                            hop_d[:, t * 2048 : (t + 1) * 2048], T2[:])
                    elif t in bset:
                        tree_push(0, lambda dst, p=ps: nc.vector.tensor_reduce(
                            dst, p[:].rearrange("p (c i) -> p i c", c=2),
                            axis=mybir.AxisListType.X, op=ALU.min), tp)
                    else:
                        G = gp.tile([128, 2048], F16, tag="g")
                        nc.scalar.activation(G[:], ps[:], ACTF.Copy)
                        tree_push(0, lambda dst, g=G: nc.vector.tensor_tensor(
                            dst, g[:, 0:1024], g[:, 1024:2048], op=ALU.min), tp)

                pend = [levels[lv][:, 0:1024]
                        for lv in sorted(levels) if levels[lv] is not None]
                nx = 0
                while len(pend) > 1:
                    a = pend.pop(0)
                    b = pend.pop(0)
                    nb = tp.tile([128, R], F16, tag=f"trx{nx}")
                    nx += 1
                    nc.vector.tensor_tensor(nb[:], a, b, op=ALU.min)
                    pend.append(nb[:])
                PRE = pend[0]
                # last dealt pair: strided reduce merged post-collapse
                ps = pd.tile([128, 2048], F32, tag="ps")
                emit_mms(ps, "d", npair - 1)
                WL = tp.tile([128, R], F16, tag="wl")
                nc.vector.tensor_reduce(
                    WL[:], ps[:].rearrange("p (c i) -> p i c", c=2),
                    axis=mybir.AxisListType.X, op=ALU.min)
                FD = tp.tile([128, R], F16, tag="fd")
                nc.vector.tensor_tensor(FD[:], PRE, WL[:], op=ALU.min)
                nc.vector.scalar_tensor_tensor(
                    ACCF[:], FD[:], SHAT[:, 0:1], DUM[:],
                    op0=ALU.add, op1=ALU.min,
                )

            with tc.tile_pool(name="fpsum", bufs=4, space=bass.MemorySpace.PSUM) as pf:
                for t in range(RT):
                    pn = pf.tile([128, 128], F16, tag="pn")
                    nc.tensor.transpose(pn[:], ACCF[:, t * 128 : (t + 1) * 128], IDN[:])
                    nc.vector.tensor_reduce(HN2[:, t : t + 1], pn[:],
                                            axis=mybir.AxisListType.X, op=ALU.min)

            nc.sync.dma_start(hn2_d[:], HN2[:])

    nc.compile()
    return nc


def _get_nc(kown):
    if kown not in _NC_CACHE:
        _NC_CACHE[kown] = _build_nc(kown)
    return _NC_CACHE[kown]


def _prep_in_maps(embeddings, labels):
    x = np.asarray(embeddings, dtype=np.float32)
    lab = np.asarray(labels).astype(np.int64)
    order = np.argsort(lab, kind="stable")
    lab_s = lab[order]
    xs = x[order]
    xt = np.ascontiguousarray(xs.T).astype(np.float16)   # [128, B]
    sq = (xs.astype(np.float64) ** 2).sum(1).astype(np.float32)
    idn = np.eye(128, dtype=np.float16)
    own_sets = []
    K = 0
    for m in range(N_CORES):
        mylab = lab_s[m * R : (m + 1) * R]
        own_idx = np.flatnonzero((lab_s >= mylab.min()) & (lab_s <= mylab.max()))
        own_sets.append(own_idx)
        K = max(K, -(-len(own_idx) // 128))
    K += K % 2
    in_maps = []
    extras = []
    for m in range(N_CORES):
        own_idx = own_sets[m]
        mask = np.zeros(B, bool)
        mask[own_idx] = True
        non_own = np.flatnonzero(~mask)
        n_fill = K * 128 - len(own_idx)
        fill, dealt = non_own[:n_fill], non_own[n_fill:]
        own_cols = np.concatenate([own_idx, fill])
        dsort = dealt[np.argsort(sq[dealt], kind="stable")]
        deal_mat = dsort.reshape(128, NCH - K)           # [partition, chunk]
        cols = np.concatenate([own_cols, deal_mat.T.reshape(-1)])
        in_maps.append({
            "xt": np.ascontiguousarray(xt[:, cols]),
            "xls": np.ascontiguousarray(
                (-2.0 * xs[m * R : (m + 1) * R].T)).astype(np.float16),
            "shat": sq[deal_mat].mean(1, dtype=np.float64).astype(np.float32).reshape(128, 1),
            "idn": idn,
        })
        extras.append(own_cols)
    return in_maps, lab, order, lab_s, sq, K, extras


def run_cores(embeddings, labels, trace=False, **kw):
    in_maps, lab, order, lab_s, sq, K, extras = _prep_in_maps(embeddings, labels)
    nc = _get_nc(K)
    res = run_bass_kernel_spmd(nc, in_maps, list(range(N_CORES)), trace=trace, **kw)
    hn2_parts, hp2_parts = [], []
    for m, r in enumerate(res.results):
        own_cols = extras[m]
        hn2 = np.asarray(r["hn2"], np.float32).T.reshape(R)
        V = np.asarray(r["hop"], np.float32).reshape(128, K // 2, 2, R)
        V = V.transpose(1, 2, 0, 3).reshape(K * 128, R)
        D2 = V + sq[own_cols][:, None]
        mylab = lab_s[m * R : (m + 1) * R]
        same = lab_s[own_cols][:, None] == mylab[None, :]
        selfm = own_cols[:, None] == (m * R + np.arange(R))[None, :]
        hn2 = np.minimum(hn2, np.where(~same, D2, np.inf).min(0))
        hp2 = np.where(same & ~selfm, D2, -np.inf).max(0)
        hn2_parts.append(hn2)
        hp2_parts.append(hp2)
    hn2 = np.concatenate(hn2_parts)
    hp2 = np.concatenate(hp2_parts)
    hn = np.sqrt(np.maximum(hn2 + sq, 0.0))
    hp = np.sqrt(np.maximum(np.where(np.isfinite(hp2), hp2, -sq) + sq, 0.0))
    pr_sorted = np.maximum(hp - hn + 1.0, 0.0)
    pr = np.empty(B, np.float32)
    pr[order] = pr_sorted
    counts = np.bincount(lab, minlength=C)
    valid = (counts[lab] >= 2) & (counts[lab] <= B - 1)
    nv = int(valid.sum())
    loss = float((pr * valid).sum() / nv) if nv > 0 else 0.0
    return np.float32(loss), res


def kernel(embeddings, labels):
    loss, _ = run_cores(embeddings, labels, trace=False)
    return loss
